# revision 12
# baseline (speedup 1.0000x reference)
"""BinaryLinear kernel for 8x TRN2 NeuronCores.

out = x @ (weight > 0)  with x [8192, 2048] f32, weight [2048, 2048] f32.

Sharding: data-parallel over batch (1024 rows/core), weight replicated.

Per core (M=1024, K=2048, N=2048). PE-matmul-bound (512 N=512 bf16
matmuls ~= 109us at 2.4GHz); everything else keeps the PE stream dense:

- Engine assignment: PE does ONLY matmuls. x transposes run on DVE as 16
  cross-quadrant 32x32 StreamTranspose instructions per batch tile
  (nch=32 DVE ops may write any partition quadrant). Weight binarize
  ((w>0)->bf16 {0,1}) runs on the otherwise-idle GPSIMD engine, as do
  the chunk re-accumulation adds; x casts and all PSUM evictions run on
  ACT.

- The weight stream (16MB f32) dominates the serial ~340GB/s DMA budget
  (~47us), while 8 PSUM banks can only absorb ~1.7us of matmul per
  arriving 1MB weight k-tile. The schedule splits the K reduction of
  early batch tiles into two chunks so banks turn over mid-stream:
  partial sums park in SBUF as bf16 (precision cost ~3e-4 rel, gate is
  2e-2) and are re-added on close. bt4..7 run as plain full-K blocks
  once all weights are resident. DMA order x0..x3, w0..w15, x4..x7
  keeps the DVE transpose chain fed while weights stream.
"""

import numpy as np

import concourse.bass as bass
import concourse.mybir as mybir
import concourse.tile as tile
from concourse import bacc
from concourse.bass_utils import run_bass_kernel_spmd

B, K, N = 8192, 2048, 2048
N_CORES = 8
MB = B // N_CORES          # 1024 batch rows per core
P = 128
KT = K // P                # 16 k-tiles
BT = MB // P               # 8 batch tiles per core
NT = 4                     # output column blocks
NB = N // NT               # 512
SQ = 32                    # DVE stream-transpose square
KH = KT // 2

F32 = mybir.dt.float32
BF16 = mybir.dt.bfloat16


def build_kernel(repeat: int = 1, mode: str = "full"):
    nc = bacc.Bacc(None, target_bir_lowering=False)
    x = nc.dram_tensor("x", [MB, K], F32, kind="ExternalInput")
    w = nc.dram_tensor("w", [K, N], F32, kind="ExternalInput")
    out = nc.dram_tensor("out", [MB, N], F32, kind="ExternalOutput")

    w3 = w[:].rearrange("(kt p) n -> p kt n", p=P)   # [128, 16, 2048]

    def body(tc, pools):
        (xraw_pool, xbf_pool, xT_pool, wraw_pool, wbin_pool,
         osb_pool, stage_pool, psum_a, psum_b) = pools
        do_x = mode in ("full", "nomm", "xonly")
        do_w = mode in ("full", "nomm", "wonly")
        do_mm = mode in ("full", "mmonly")

        xbf = {}
        xT = {}
        wbin = {}
        osb = {}

        def emit_x_load(bt):
            xr = xraw_pool.tile([P, K], F32, tag="xraw", name="xr")
            xb = xbf_pool.tile([P, K], BF16, tag=f"xbf_{bt % 3}",
                               name=f"xbf_{bt}")
            nc.sync.dma_start(xr[:], x[bt * P:(bt + 1) * P, :])
            for c in range(4):
                nc.scalar.activation(
                    xb[:, c * NB:(c + 1) * NB], xr[:, c * NB:(c + 1) * NB],
                    mybir.ActivationFunctionType.Copy)
            xbf[bt] = xb

        def emit_transpose(bt, halves=1):
            """DVE cross-quadrant stream transpose of x tile bt into
            xT[bt] ([128, 16*128] bf16; block kt = x block kt transposed).
            halves=2 splits by kt-halves to cut first-use latency."""
            t = xT_pool.tile([P, K], BF16, tag=f"xT_{bt}", name=f"xT_{bt}")
            if do_x:
                src = xbf[bt]
                for h in range(halves):
                    hk = KT // halves
                    for i in range(4):
                        for q in range(4):
                            in_ap = src[SQ * i:SQ * (i + 1),
                                        h * hk * P:(h + 1) * hk * P].rearrange(
                                "p (kt c) -> p kt c", kt=hk)[
                                :, :, SQ * q:SQ * (q + 1)]
                            out_ap = t[SQ * q:SQ * (q + 1),
                                       h * hk * P:(h + 1) * hk * P].rearrange(
                                "p (kt c) -> p kt c", kt=hk)[
                                :, :, SQ * i:SQ * (i + 1)]
                            nc.vector.transpose(out_ap, in_ap)
            else:
                nc.any.memset(t[:], 1.0)
            xT[bt] = t

        def emit_w_load(kt):
            wb = wbin_pool.tile([P, N], BF16, tag=f"wbin_{kt}",
                                name=f"wbin_{kt}")
            if do_w:
                wr = wraw_pool.tile([P, N], F32, tag="wraw", name="wr")
                nc.sync.dma_start(wr[:], w3[:, kt, :])
                for c in range(2):
                    eng = nc.vector if (kt >= KH and c == 1) else nc.gpsimd
                    eng.tensor_scalar(
                        out=wb[:, c * (N // 2):(c + 1) * (N // 2)],
                        in0=wr[:, c * (N // 2):(c + 1) * (N // 2)],
                        scalar1=0.0, scalar2=None,
                        op0=mybir.AluOpType.is_gt)
            else:
                nc.any.memset(wb[:], 1.0)
            wbin[kt] = wb

        # ---- DMA order: x0..x3 ; w0..w15 ; x4..x7 ----
        if do_x:
            for bt in range(4):
                emit_x_load(bt)
        for kt in range(KT):
            emit_w_load(kt)
        if do_x:
            for bt in range(4, BT):
                emit_x_load(bt)

        emit_transpose(0, halves=4)
        emit_transpose(1, halves=2)
        emit_transpose(2, halves=2)
        emit_transpose(3, halves=2)

        # ---- wave machinery: each wave = bt x nt0..3 in 4 PSUM banks.
        # Two 4-bank pools: "a" for long-held stream waves, "b" for the
        # fillers cycling underneath them, so a filler never lands on a
        # bank held by a wave that closes later in program order. ----
        waves = {}

        def wv_open(tag, bt, pool):
            pp = psum_a if pool == "a" else psum_b
            waves[tag] = (bt, {nt: pp.tile([P, NB], F32, tag=f"ps{pool}",
                                           name="ps")
                               for nt in range(NT)}, [0])

        def wv_mm(tag, kts, last=False):
            bt, pss, cnt = waves[tag]
            for kt in kts:
                cnt[0] += 1
                for nt in range(NT):
                    nc.tensor.matmul(
                        pss[nt][:],
                        xT[bt][:, kt * P:(kt + 1) * P],
                        wbin[kt][:, nt * NB:(nt + 1) * NB],
                        start=(cnt[0] == 1), stop=last and kt == kts[-1])

        def wv_close(tag, mode_):
            """c1: park partial as bf16 in SBUF.
            c2: stage=psum (ACT), stage+=osb (Pool), DMA out.
            full: stage=psum (ACT), DMA out."""
            bt, pss, cnt = waves.pop(tag)
            if mode_ == "c1":
                ob = osb_pool.tile([P, N], BF16, tag=f"osb_{bt}",
                                   name=f"osb_{bt}")
                for nt in range(NT):
                    nc.scalar.activation(
                        ob[:, nt * NB:(nt + 1) * NB], pss[nt][:],
                        mybir.ActivationFunctionType.Copy)
                osb[bt] = ob
                return
            st = stage_pool.tile([P, N], F32, tag="stage", name="st")
            for nt in range(NT):
                nc.scalar.activation(
                    st[:, nt * NB:(nt + 1) * NB], pss[nt][:],
                    mybir.ActivationFunctionType.Copy)
                if mode_ == "c2":
                    nc.gpsimd.tensor_tensor(
                        out=st[:, nt * NB:(nt + 1) * NB],
                        in0=st[:, nt * NB:(nt + 1) * NB],
                        in1=osb[bt][:, nt * NB:(nt + 1) * NB],
                        op=mybir.AluOpType.add)
                if nt % 2 == 1:
                    h = nt // 2
                    nc.sync.dma_start(
                        out[bt * P:(bt + 1) * P,
                            h * (N // 2):(h + 1) * (N // 2)],
                        st[:, h * (N // 2):(h + 1) * (N // 2)])

        if do_mm:
            # intro: kt-major chunk1 for bt0+bt1, bt1 lagging 2 kts
            wv_open("I0", 0, "a")
            wv_open("I1", 1, "b")
            for s in range(KH + 2):
                if s < KH:
                    wv_mm("I0", [s], last=(s == KH - 1))
                if s >= 2:
                    wv_mm("I1", [s - 2], last=(s - 2 == KH - 1))
            wv_close("I0", "c1")
            wv_close("I1", "c1")

            # stream era: bt2 chunkA (pool a) rides the late-weight
            # arrivals; filler waves (pool b) run FIRST in program order
            # so the in-order PE never head-of-line blocks on a paced kt
            # while resident-kt work exists.
            wv_open("A2", 2, "a")
            wv_mm("A2", [8, 9])
            wv_open("B2", 2, "b")
            wv_mm("B2", [0, 1, 2, 3, 4, 5, 6, 7], last=True)
            wv_close("B2", "c1")
            wv_open("B3", 3, "b")
            wv_mm("B3", [0, 1, 2, 3, 4, 5, 6, 7], last=True)
            wv_close("B3", "c1")
            wv_open("C0", 0, "b")
            wv_mm("C0", [8, 9, 10, 11, 12])
            wv_mm("A2", [10, 11, 12, 13])
            wv_mm("C0", [13])
            wv_mm("A2", [14, 15], last=True)
            wv_close("A2", "c2")
            wv_mm("C0", [14, 15], last=True)
            wv_close("C0", "c2")

            # free-run: all weights resident
            for tag, bt, kts, pl in (("C1", 1, list(range(KH, KT)), "a"),
                                     ("C3", 3, list(range(KH, KT)), "b")):
                wv_open(tag, bt, pl)
                wv_mm(tag, kts, last=True)
                wv_close(tag, "c2")
            for bt in range(4, BT - 1):
                emit_transpose(bt)
                tag = f"F{bt}"
                wv_open(tag, bt, "a" if bt % 2 == 0 else "b")
                wv_mm(tag, list(range(KT)), last=True)
                wv_close(tag, "full")
            # last tile: two nt-pair half-waves so the first half's
            # eviction + out-DMA pipeline under the second half's matmuls
            emit_transpose(BT - 1)
            st_last = stage_pool.tile([P, N], F32, tag="stage", name="st")
            for gi, nts in enumerate(((0, 1), (2,), (3,))):
                pp = {nt: (psum_b if gi % 2 else psum_a).tile(
                    [P, NB], F32, tag="psb" if gi % 2 else "psa", name="ps")
                    for nt in nts}
                for kt in range(KT):
                    for nt in nts:
                        nc.tensor.matmul(
                            pp[nt][:],
                            xT[BT - 1][:, kt * P:(kt + 1) * P],
                            wbin[kt][:, nt * NB:(nt + 1) * NB],
                            start=(kt == 0), stop=(kt == KT - 1))
                for nt in nts:
                    nc.scalar.activation(
                        st_last[:, nt * NB:(nt + 1) * NB], pp[nt][:],
                        mybir.ActivationFunctionType.Copy)
                    nc.sync.dma_start(
                        out[(BT - 1) * P:BT * P, nt * NB:(nt + 1) * NB],
                        st_last[:, nt * NB:(nt + 1) * NB])
        else:
            for bt in range(4, BT):
                emit_transpose(bt)

    with tile.TileContext(nc) as tc:
        with (
            tc.tile_pool(name="xraw", bufs=2) as xraw_pool,
            tc.tile_pool(name="xbf", bufs=1) as xbf_pool,
            tc.tile_pool(name="xT", bufs=1) as xT_pool,
            tc.tile_pool(name="wraw", bufs=4) as wraw_pool,
            tc.tile_pool(name="wbin", bufs=1) as wbin_pool,
            tc.tile_pool(name="osb", bufs=1) as osb_pool,
            tc.tile_pool(name="stage", bufs=3) as stage_pool,
            tc.tile_pool(name="psa", bufs=4, space="PSUM") as psum_a,
            tc.tile_pool(name="psb", bufs=4, space="PSUM") as psum_b,
        ):
            pools = (xraw_pool, xbf_pool, xT_pool, wraw_pool, wbin_pool,
                     osb_pool, stage_pool, psum_a, psum_b)
            if repeat == 1:
                body(tc, pools)
            else:
                with tc.For_i(0, repeat, 1):
                    body(tc, pools)
    _dedup_ldweights(nc)
    nc.compile()
    return nc


def _ldw_key(ins):
    ap = ins.ins[0]
    bap = getattr(ap, "bass_ap", None)
    return (getattr(ap, "memref", None), getattr(bap, "offset", None),
            str(getattr(bap, "ap", None)), getattr(ins, "is_transpose", None))


def _dedup_ldweights(nc):
    """Remove PE weight reloads of the already-loaded stationary operand.

    tile_legalize emits one InstLdweights per InstMatmult even when
    consecutive matmuls share the stationary tile. An InstLdweights whose
    weights AP equals the previous load (with no intervening load) is a
    no-op on the array state and can be dropped. Only waitless/updateless
    loads are dropped, so no synchronization is lost."""
    removed = 0
    for bb in nc.main_func.blocks:
        il = bb.instructions
        last_key = None
        drop = []
        for idx, ins in enumerate(il):
            if not isinstance(ins, mybir.InstLdweights):
                continue
            si = ins.sync_info
            has_sync = si is not None and (
                (si.on_wait and len(si.on_wait) > 0)
                or (si.on_update and len(si.on_update) > 0))
            key = _ldw_key(ins)
            if key == last_key and not has_sync:
                drop.append(idx)
                removed += 1
            else:
                last_key = key
        for idx in reversed(drop):
            del il[idx]
    return removed


_NC_CACHE = None


def _get_nc():
    global _NC_CACHE
    if _NC_CACHE is None:
        _NC_CACHE = build_kernel()
    return _NC_CACHE


def kernel(x: np.ndarray, weight: np.ndarray):
    assert x.shape == (B, K) and weight.shape == (K, N)
    x = np.ascontiguousarray(x, dtype=np.float32)
    weight = np.ascontiguousarray(weight, dtype=np.float32)
    nc = _get_nc()
    in_maps = [
        {"x": x[i * MB:(i + 1) * MB], "w": weight}
        for i in range(N_CORES)
    ]
    res = run_bass_kernel_spmd(nc, in_maps, core_ids=list(range(N_CORES)))
    return np.concatenate([res.results[i]["out"] for i in range(N_CORES)], axis=0)
